# revision 1
# baseline (speedup 1.0000x reference)
"""Trainium2 Bass kernel for nn_CrossAttention (linear/efficient attention).

Math: out = x + reshape( x_flat @ W_eff + bo ) where
  W_eff = I + sum_h Wq_h @ cm_h @ Wo_h,
  cm_h  = softmax_n(k_h)^T @ v_h,  k = ctx_flat @ Wk, v = ctx_flat @ Wv.
(The q projection folds into W_eff; the residual folds in as the identity.)

Sharding: 8 cores = 4 batches x 2 token-halves. Each core computes partial
[num|den] softmax statistics over its 8192 tokens; a pairwise AllReduce
merges them; each core then applies W_eff to its own token half.
"""

import sys

if "/opt/trn_rl_repo" not in sys.path:
    sys.path.insert(0, "/opt/trn_rl_repo")

import numpy as np

B = 4
C = 256          # channels (DIM)
N_FULL = 16384   # tokens per batch (128*128)
T = 8192         # tokens per core
HEADS = 8
DH = 64
INNER = 512
NCORES = 8
CHUNK = 1024
NCH = T // CHUNK      # 8
SUBS = CHUNK // 128   # 8

_CACHE: dict = {}
LAST_RESULTS = None   # BassKernelResults of the most recent run (for profiling)
TRACE = False         # set True before calling kernel() to capture a trace


def _build_nc():
    import concourse.mybir as mybir
    import concourse.tile as tile
    from concourse import bacc
    from concourse.masks import make_identity

    f32, f16 = mybir.dt.float32, mybir.dt.float16
    AF = mybir.ActivationFunctionType

    nc = bacc.Bacc("TRN2", target_bir_lowering=False, debug=False)

    xh = nc.dram_tensor("xh", [C, T], f32, kind="ExternalInput")
    ch = nc.dram_tensor("ch", [C, T], f32, kind="ExternalInput")
    wk = nc.dram_tensor("wk", [C, INNER], f32, kind="ExternalInput")
    wv = nc.dram_tensor("wv", [C, INNER], f32, kind="ExternalInput")
    wqt = nc.dram_tensor("wqt", [INNER, C], f32, kind="ExternalInput")
    wo = nc.dram_tensor("wo", [INNER, C], f32, kind="ExternalInput")
    bo = nc.dram_tensor("bo", [C, 1], f32, kind="ExternalInput")
    out = nc.dram_tensor("out", [C, T], f32, kind="ExternalOutput")

    xh_r = xh.ap().rearrange("(kc p) n -> p kc n", p=128)
    ch_r = ch.ap().rearrange("(kc p) n -> p kc n", p=128)
    out_r = out.ap().rearrange("(oc p) n -> p oc n", p=128)

    with tile.TileContext(nc) as tc:
        with (
            tc.tile_pool(name="wpool", bufs=1) as wpool,
            tc.tile_pool(name="spool", bufs=3) as spool,
            tc.tile_pool(name="x16pool", bufs=1) as x16pool,
            tc.tile_pool(name="dpool", bufs=1, space="DRAM") as dpool,
            tc.tile_pool(name="ps_kv", bufs=4, space="PSUM") as ps_kv,
        ):
            def load_ctx16(ci):
                """HWDGE f32 load + DVE cast; returns the fp16 tile."""
                ctx_f = spool.tile([128, 2, CHUNK], f32, tag="ctx_f", name="ctx_f")
                nc.sync.dma_start(
                    ctx_f[:], ch_r[:, :, ci * CHUNK : (ci + 1) * CHUNK]
                )
                ctx16 = spool.tile([128, 2, CHUNK], f16, tag="ctx16", name="ctx16")
                nc.vector.tensor_copy(ctx16[:], ctx_f[:])
                return ctx16

            # first ctx chunk ahead of everything: the first matmul needs only
            # this and wk16
            ctx16_first = load_ctx16(0)

            # ---- weights: HWDGE f32 loads + one-time on-chip casts ----
            wk_sb = wpool.tile([128, 2, INNER], f32)
            nc.sync.dma_start(wk_sb[:], wk.ap().rearrange("(kc p) o -> p kc o", p=128))
            wk16 = wpool.tile([128, 2, INNER], f16)
            nc.vector.tensor_copy(wk16[:], wk_sb[:])
            wv_sb = wpool.tile([128, 2, INNER], f32)
            nc.sync.dma_start(wv_sb[:], wv.ap().rearrange("(kc p) o -> p kc o", p=128))
            wv16 = wpool.tile([128, 2, INNER], f16)
            nc.vector.tensor_copy(wv16[:], wv_sb[:])
            wqt_sb = wpool.tile([128, 4, C], f32)
            nc.sync.dma_start(
                wqt_sb[:], wqt.ap().rearrange("(hc p) i -> p hc i", p=128)
            )
            wqt16 = wpool.tile([128, 4, C], f16)
            nc.vector.tensor_copy(wqt16[:], wqt_sb[:])
            wo_sb = wpool.tile([64, HEADS, C], f32)
            nc.sync.dma_start(wo_sb[:], wo.ap().rearrange("(h p) o -> p h o", p=64))
            wo16 = wpool.tile([64, HEADS, C], f16)
            nc.vector.tensor_copy(wo16[:], wo_sb[:])
            bo_sb = wpool.tile([128, 2], f32)
            nc.sync.dma_start(bo_sb[:], bo.ap().rearrange("(oc p) x -> p (oc x)", p=128))
            ident16 = wpool.tile([128, 128], f16)
            make_identity(nc, ident16[:])

            # ---- phase 1: accumulate per-head [num | den] over local tokens ----
            # cm_ps[hp] rows 0:64   = head 2hp   : cols 0:64 num, col 64 den
            #           rows 64:128 = head 2hp+1 : cols 65:129 num, col 129 den
            cm_sb = wpool.tile([128, 4, 130], f32)
            x16_tiles = []

            with tc.tile_pool(name="ps_cm", bufs=1, space="PSUM") as ps_cm:
                cm_ps = [
                    ps_cm.tile([128, 130], f32, tag=f"cm{i}", name=f"cm{i}")
                    for i in range(4)
                ]
                ctx16_next = ctx16_first
                for ci in range(NCH):
                    ctx16 = ctx16_next
                    if ci + 1 < NCH:
                        # one-chunk lookahead keeps the DVE cast off the
                        # critical path of the next chunk's matmuls
                        ctx16_next = load_ctx16(ci + 1)
                    # phase-2 x tile: f32 load now, cast on ScalarE (spread
                    # through phase 1; ready before phase 2 needs it)
                    x_f = spool.tile([128, 2, CHUNK], f32, tag="x_f", name="x_f")
                    nc.sync.dma_start(
                        x_f[:], xh_r[:, :, ci * CHUNK : (ci + 1) * CHUNK]
                    )
                    x16 = x16pool.tile(
                        [128, 2, CHUNK], f16, tag=f"x16_{ci}", name=f"x16_{ci}"
                    )
                    # alternate the cast between ScalarE and VectorE so neither
                    # becomes the phase-1 straggler
                    if ci % 2 == 0:
                        nc.scalar.copy(x16[:], x_f[:])
                    else:
                        nc.vector.tensor_copy(x16[:], x_f[:])
                    x16_tiles.append(x16)

                    for s in range(SUBS):
                        tok = slice(s * 128, (s + 1) * 128)
                        k_ps = ps_kv.tile([128, INNER], f32, tag="kv")
                        for kc in range(2):
                            nc.tensor.matmul(
                                k_ps[:],
                                lhsT=ctx16[:, kc, tok],
                                rhs=wk16[:, kc, :],
                                start=(kc == 0),
                                stop=(kc == 1),
                            )
                        kexp = spool.tile([128, INNER], f16, tag="kexp")
                        nc.scalar.activation(kexp[:], k_ps[:], AF.Exp)
                        v_ps = ps_kv.tile([128, INNER], f32, tag="kv")
                        for kc in range(2):
                            nc.tensor.matmul(
                                v_ps[:],
                                lhsT=ctx16[:, kc, tok],
                                rhs=wv16[:, kc, :],
                                start=(kc == 0),
                                stop=(kc == 1),
                            )
                        vcat = spool.tile([128, 8, 65], f16, tag="vcat")
                        nc.vector.tensor_copy(
                            vcat[:, :, 0:64],
                            v_ps[:].rearrange("p (h e) -> p h e", h=8),
                        )
                        nc.gpsimd.memset(vcat[:, :, 64], 1.0)
                        first = ci == 0 and s == 0
                        last = ci == NCH - 1 and s == SUBS - 1
                        for hp in range(4):
                            nc.tensor.matmul(
                                cm_ps[hp][:],
                                lhsT=kexp[:, hp * 128 : (hp + 1) * 128],
                                rhs=vcat[:, 2 * hp : 2 * hp + 2, :],
                                start=first,
                                stop=last,
                            )
                for hp in range(4):
                    nc.vector.tensor_copy(cm_sb[:, hp, :], cm_ps[hp][:])

            # ---- pairwise AllReduce of [num|den] across the 2 token halves ----
            # trim to the useful halves: head h -> rows (h%2)*64, chunk h//2,
            # cols 0:64 num, col 64 den
            cc_in = dpool.tile([128, 4, 65], f32)
            cc_out = dpool.tile([128, 4, 65], f32)
            nc.sync.dma_start(cc_in[0:64, :, :], cm_sb[0:64, :, 0:65])
            nc.sync.dma_start(cc_in[64:128, :, :], cm_sb[64:128, :, 65:130])
            nc.gpsimd.collective_compute(
                "AllReduce",
                mybir.AluOpType.add,
                replica_groups=[[0, 1], [2, 3], [4, 5], [6, 7]],
                ins=[cc_in.opt()],
                outs=[cc_out.opt()],
            )
            mm_sb = wpool.tile([128, 4, 65], f32)
            nc.sync.dma_start(mm_sb[:], cc_out[:])

            # keep the PE clock warm through the AllReduce window
            warm_ps = ps_kv.tile([128, INNER], f32, tag="kv", name="warm_ps")
            for _ in range(110):
                nc.tensor.matmul(
                    warm_ps[:],
                    lhsT=wk16[:, 0, 0:128],
                    rhs=wk16[:, 1, :],
                    start=True,
                    stop=True,
                )

            # ---- normalize cm, build W_eff = I + sum_h Wq_h cm_h Wo_h ----
            deninv = wpool.tile([128, 4], f32)
            cmn16 = wpool.tile([128, 4, 64], f16)
            m1t16 = wpool.tile([64, 8, C], f16)
            weff16 = wpool.tile([128, 2, C], f16)
            with tc.tile_pool(name="ps_post", bufs=2, space="PSUM") as ps_post:
                nc.vector.reciprocal(deninv[:], mm_sb[:, :, 64])
                for hp in range(4):
                    nc.vector.tensor_scalar_mul(
                        cmn16[:, hp, :],
                        mm_sb[:, hp, 0:64],
                        deninv[:, hp : hp + 1],
                    )
                for h in range(HEADS):
                    hp, hh = h // 2, h % 2
                    rs = slice(hh * 64, hh * 64 + 64)
                    m1t_ps = ps_post.tile([64, C], f32, tag="m1t")
                    nc.tensor.matmul(
                        m1t_ps[:],
                        lhsT=cmn16[rs, hp, :],
                        rhs=wqt16[rs, hp, :],
                        start=True,
                        stop=True,
                    )
                    nc.vector.tensor_copy(m1t16[:, h, :], m1t_ps[:])
                for ic in range(2):
                    weff_ps = ps_post.tile([128, C], f32, tag="weff")
                    for h in range(HEADS):
                        nc.tensor.matmul(
                            weff_ps[:],
                            lhsT=m1t16[:, h, ic * 128 : (ic + 1) * 128],
                            rhs=wo16[:, h, :],
                            start=(h == 0),
                            stop=False,
                        )
                    # fold the residual in: W_eff += I (this core's row block)
                    nc.tensor.matmul(
                        weff_ps[:, ic * 128 : (ic + 1) * 128],
                        lhsT=ident16[:],
                        rhs=ident16[:],
                        start=False,
                        stop=True,
                    )
                    nc.vector.tensor_copy(weff16[:, ic, :], weff_ps[:])

            # ---- phase 2: out = W_eff^T @ x (+bo), token-major native layout ----
            for ci in range(NCH):
                x16 = x16_tiles[ci]
                out_sb = spool.tile([128, 2, CHUNK], f32, tag="out_sb")
                for oc in range(2):
                    for nh in range(2):
                        ts_ = slice(nh * 512, (nh + 1) * 512)
                        o_ps = ps_kv.tile([128, INNER], f32, tag="kv", name="o_ps")
                        for ic in range(2):
                            nc.tensor.matmul(
                                o_ps[:],
                                lhsT=weff16[:, ic, oc * 128 : (oc + 1) * 128],
                                rhs=x16[:, ic, ts_],
                                start=(ic == 0),
                                stop=(ic == 1),
                            )
                        # bias+copy: alternate ACT / DVE to balance engines
                        if nh == 0:
                            nc.scalar.activation(
                                out_sb[:, oc, ts_],
                                o_ps[:],
                                AF.Identity,
                                bias=bo_sb[:, oc : oc + 1],
                            )
                        else:
                            nc.vector.tensor_scalar_add(
                                out_sb[:, oc, ts_], o_ps[:], bo_sb[:, oc : oc + 1]
                            )
                nc.sync.dma_start(out_r[:, :, ci * CHUNK : (ci + 1) * CHUNK], out_sb[:])

    nc.compile()
    return nc


def _get_nc():
    if "nc" not in _CACHE:
        _CACHE["nc"] = _build_nc()
    return _CACHE["nc"]


def kernel(**inputs) -> np.ndarray:
    global LAST_RESULTS
    from concourse.bass_utils import run_bass_kernel_spmd

    x = np.ascontiguousarray(np.asarray(inputs["x"], dtype=np.float32))
    ctx = np.ascontiguousarray(np.asarray(inputs["context"], dtype=np.float32))
    Wq = np.asarray(inputs["Wq"], dtype=np.float32)
    Wk = np.ascontiguousarray(np.asarray(inputs["Wk"], dtype=np.float32))
    Wv = np.ascontiguousarray(np.asarray(inputs["Wv"], dtype=np.float32))
    Wo = np.ascontiguousarray(np.asarray(inputs["Wo"], dtype=np.float32))
    bo = np.ascontiguousarray(
        np.asarray(inputs["bo"], dtype=np.float32).reshape(C, 1)
    )
    wqt = np.ascontiguousarray(Wq.T)

    xf = x.reshape(B, C, N_FULL)
    cf = ctx.reshape(B, C, N_FULL)

    in_maps = []
    for c in range(NCORES):
        b, t = c // 2, c % 2
        sl = slice(t * T, (t + 1) * T)
        in_maps.append(
            {
                "xh": np.ascontiguousarray(xf[b, :, sl]),
                "ch": np.ascontiguousarray(cf[b, :, sl]),
                "wk": Wk,
                "wv": Wv,
                "wqt": wqt,
                "wo": Wo,
                "bo": bo,
            }
        )

    nc = _get_nc()
    res = run_bass_kernel_spmd(nc, in_maps, list(range(NCORES)), trace=TRACE)
    LAST_RESULTS = res

    out = np.empty((B, C, N_FULL), dtype=np.float32)
    for c in range(NCORES):
        b, t = c // 2, c % 2
        out[b, :, t * T : (t + 1) * T] = res.results[c]["out"]
    return out.reshape(B, C, 128, 128)



# revision 8
# speedup vs baseline: 1.0724x; 1.0724x over previous
"""Trainium2 Bass kernel for nn_CrossAttention (linear/efficient attention).

Math: out = x + reshape( x_flat @ W_eff + bo ) where
  W_eff = I + sum_h Wq_h @ cm_h @ Wo_h,
  cm_h  = softmax_n(k_h)^T @ v_h,  k = ctx_flat @ Wk, v = ctx_flat @ Wv.
(The q projection folds into W_eff; the residual folds in as the identity.)

Sharding: 8 cores = 4 batches x 2 token-halves. Each core computes partial
[num|den] softmax statistics over its 8192 tokens; a pairwise AllReduce
merges them; each core then applies W_eff to its own token half.

v2: phase-1 projections and cm accumulation run in fp8 (DoubleRow perf
mode, 256-deep contraction per pass); ctx arrives fp8 and x fp16 from the
host; the output is stored fp16 and widened on the host. Phase 2 stays
fp16 end to end so the residual identity inside W_eff keeps x's accuracy.
"""

import sys

if "/opt/trn_rl_repo" not in sys.path:
    sys.path.insert(0, "/opt/trn_rl_repo")

import numpy as np
import ml_dtypes

B = 4
C = 256          # channels (DIM)
N_FULL = 16384   # tokens per batch (128*128)
T = 8192         # tokens per core
HEADS = 8
DH = 64
INNER = 512
NCORES = 8
CHUNK = 2048
NCH = T // CHUNK      # 4
SUBS = CHUNK // 128   # 16

_CACHE: dict = {}
LAST_RESULTS = None   # BassKernelResults of the most recent run (for profiling)
TRACE = False         # set True before calling kernel() to capture a trace


def _build_nc():
    import concourse.mybir as mybir
    import concourse.tile as tile
    from concourse import bacc
    from concourse.masks import make_identity

    f32, f16, f8 = mybir.dt.float32, mybir.dt.float16, mybir.dt.float8e4
    AF = mybir.ActivationFunctionType
    DR = mybir.MatmulPerfMode.DoubleRow

    nc = bacc.Bacc("TRN2", target_bir_lowering=False, debug=False)

    xh = nc.dram_tensor("xh", [C, T], f16, kind="ExternalInput")
    ch = nc.dram_tensor("ch", [C, T], f8, kind="ExternalInput")
    wkv = nc.dram_tensor("wkv", [C, 2 * INNER], f8, kind="ExternalInput")
    wqt = nc.dram_tensor("wqt", [INNER, C], f16, kind="ExternalInput")
    wo = nc.dram_tensor("wo", [INNER, C], f16, kind="ExternalInput")
    bo = nc.dram_tensor("bo", [C, 1], f32, kind="ExternalInput")
    out = nc.dram_tensor("out", [C, T], f16, kind="ExternalOutput")

    xh_r = xh.ap().rearrange("(kc p) n -> p kc n", p=128)
    ch_r = ch.ap().rearrange("(kc p) n -> p kc n", p=128)
    out_r = out.ap().rearrange("(oc p) n -> p oc n", p=128)

    with tile.TileContext(nc) as tc:
        with (
            tc.tile_pool(name="wpool", bufs=1) as wpool,
            tc.tile_pool(name="spool", bufs=3) as spool,
            tc.tile_pool(name="ppool", bufs=2) as ppool,
            tc.tile_pool(name="x16pool", bufs=1) as x16pool,
            tc.tile_pool(name="dpool", bufs=1, space="DRAM") as dpool,
        ):
            def load_ctx8(ci):
                ctx8 = spool.tile([128, 2, CHUNK], f8, tag="ctx8", name="ctx8")
                nc.sync.dma_start(
                    ctx8[:], ch_r[:, :, ci * CHUNK : (ci + 1) * CHUNK]
                )
                return ctx8

            ctx8_next = load_ctx8(0)

            # ---- weights (already converted on host) ----
            wkv8 = wpool.tile([128, 2, 2 * INNER], f8)
            nc.sync.dma_start(wkv8[:], wkv.ap().rearrange("(kc p) o -> p kc o", p=128))
            wqt16 = wpool.tile([128, 4, C], f16)
            nc.sync.dma_start(
                wqt16[:], wqt.ap().rearrange("(hc p) i -> p hc i", p=128)
            )
            wo16 = wpool.tile([64, HEADS, C], f16)
            nc.sync.dma_start(wo16[:], wo.ap().rearrange("(h p) o -> p h o", p=64))
            bo_sb = wpool.tile([128, 2], f32)
            nc.sync.dma_start(bo_sb[:], bo.ap().rearrange("(oc p) x -> p (oc x)", p=128))
            ident16 = wpool.tile([128, 128], f16)
            make_identity(nc, ident16[:])

            # ---- phase 1: accumulate per-head [num | den] over local tokens ----
            # cm_ps[hp] rows 0:64   = head 2hp   : cols 0:64 num, col 64 den
            #           rows 64:128 = head 2hp+1 : cols 65:129 num, col 129 den
            cm_sb = wpool.tile([128, 4, 130], f32)
            x16_tiles = []

            with (
                tc.tile_pool(name="ps_cm", bufs=1, space="PSUM") as ps_cm,
                tc.tile_pool(name="ps_kv", bufs=2, space="PSUM") as ps_kv,
            ):
                cm_ps = [
                    ps_cm.tile([128, 130], f32, tag=f"cm{i}", name=f"cm{i}")
                    for i in range(4)
                ]
                for ci in range(NCH):
                    ctx8 = ctx8_next
                    if ci + 1 < NCH:
                        ctx8_next = load_ctx8(ci + 1)
                    # phase-2 x tile: f16 straight from HBM, kept resident
                    x16 = x16pool.tile(
                        [128, 2, CHUNK], f16, tag=f"x16_{ci}", name=f"x16_{ci}"
                    )
                    nc.sync.dma_start(
                        x16[:], xh_r[:, :, ci * CHUNK : (ci + 1) * CHUNK]
                    )
                    x16_tiles.append(x16)

                    for s in range(SUBS):
                        tok = slice(s * 128, (s + 1) * 128)
                        half = s % 2
                        if half == 0:
                            kexp8 = ppool.tile([128, 2, INNER], f8, tag="kexp")
                            vcat8 = ppool.tile([128, 2, 8, 65], f8, tag="vcat")
                            nc.gpsimd.memset(vcat8[:, :, :, 64], 1.0)
                        # K and V projections: contraction 256 in one
                        # DoubleRow pass each (PSUM bank limits out to 512 f32)
                        k_ps = ps_kv.tile([128, INNER], f32, tag="k")
                        nc.tensor.matmul(
                            k_ps[:],
                            lhsT=ctx8[:, :, tok],
                            rhs=wkv8[:, :, 0:INNER],
                            start=True,
                            stop=True,
                            perf_mode=DR,
                        )
                        v_ps = ps_kv.tile([128, INNER], f32, tag="v")
                        nc.tensor.matmul(
                            v_ps[:],
                            lhsT=ctx8[:, :, tok],
                            rhs=wkv8[:, :, INNER : 2 * INNER],
                            start=True,
                            stop=True,
                            perf_mode=DR,
                        )
                        nc.scalar.activation(
                            kexp8[:, half, :], k_ps[:], AF.Exp
                        )
                        nc.vector.tensor_copy(
                            vcat8[:, half, :, 0:64],
                            v_ps[:].rearrange("p (h e) -> p h e", h=8),
                        )
                        if half == 1:
                            first = ci == 0 and s == 1
                            last = ci == NCH - 1 and s == SUBS - 1
                            for hp in range(4):
                                nc.tensor.matmul(
                                    cm_ps[hp][:],
                                    lhsT=kexp8[:, :, hp * 128 : (hp + 1) * 128],
                                    rhs=vcat8[:, :, 2 * hp : 2 * hp + 2, :],
                                    start=first,
                                    stop=last,
                                    perf_mode=DR,
                                )
                for hp in range(4):
                    nc.vector.tensor_copy(cm_sb[:, hp, :], cm_ps[hp][:])

            # ---- pairwise AllReduce of [num|den] across the 2 token halves ----
            # trim to the useful halves: head h -> rows (h%2)*64, chunk h//2,
            # cols 0:64 num, col 64 den
            cc_in = dpool.tile([128, 4, 65], f32)
            cc_out = dpool.tile([128, 4, 65], f32)
            nc.sync.dma_start(cc_in[0:64, :, :], cm_sb[0:64, :, 0:65])
            nc.sync.dma_start(cc_in[64:128, :, :], cm_sb[64:128, :, 65:130])
            nc.gpsimd.collective_compute(
                "AllReduce",
                mybir.AluOpType.add,
                replica_groups=[[0, 1], [2, 3], [4, 5], [6, 7]],
                ins=[cc_in.opt()],
                outs=[cc_out.opt()],
            )
            mm_sb = wpool.tile([128, 4, 65], f32)
            nc.sync.dma_start(mm_sb[:], cc_out[:])

            # keep the PE clock warm through the AllReduce window: small
            # matmuls that drain fast once real work arrives
            with (
                tc.tile_pool(name="ps_warm", bufs=1, space="PSUM") as ps_warm,
                tc.tile_pool(name="ps_post", bufs=2, space="PSUM") as ps_post,
            ):
                warm_ps = ps_warm.tile([128, C], f32, tag="warm", name="warm_ps")
                for _ in range(56):
                    nc.tensor.matmul(
                        warm_ps[:],
                        lhsT=wqt16[:, 0, 0:128],
                        rhs=wqt16[:, 1, :],
                        start=True,
                        stop=True,
                    )

                # ---- normalize cm, build W_eff = I + sum_h Wq_h cm_h Wo_h ----
                deninv = wpool.tile([128, 4], f32)
                cmn16 = wpool.tile([128, 4, 64], f16)
                m1t16 = wpool.tile([64, 8, C], f16)
                weff16 = wpool.tile([128, 2, C], f16)
                nc.vector.reciprocal(deninv[:], mm_sb[:, :, 64])
                for hp in range(4):
                    nc.vector.tensor_scalar_mul(
                        cmn16[:, hp, :],
                        mm_sb[:, hp, 0:64],
                        deninv[:, hp : hp + 1],
                    )
                for h in range(HEADS):
                    hp, hh = h // 2, h % 2
                    rs = slice(hh * 64, hh * 64 + 64)
                    m1t_ps = ps_post.tile([64, C], f32, tag="m1t")
                    nc.tensor.matmul(
                        m1t_ps[:],
                        lhsT=cmn16[rs, hp, :],
                        rhs=wqt16[rs, hp, :],
                        start=True,
                        stop=True,
                    )
                    nc.vector.tensor_copy(m1t16[:, h, :], m1t_ps[:])
                for ic in range(2):
                    weff_ps = ps_post.tile([128, C], f32, tag="weff")
                    for h in range(HEADS):
                        nc.tensor.matmul(
                            weff_ps[:],
                            lhsT=m1t16[:, h, ic * 128 : (ic + 1) * 128],
                            rhs=wo16[:, h, :],
                            start=(h == 0),
                            stop=False,
                        )
                    # fold the residual in: W_eff += I (this core's row block)
                    nc.tensor.matmul(
                        weff_ps[:, ic * 128 : (ic + 1) * 128],
                        lhsT=ident16[:],
                        rhs=ident16[:],
                        start=False,
                        stop=True,
                    )
                    nc.vector.tensor_copy(weff16[:, ic, :], weff_ps[:])

                # ---- phase 2: out = W_eff^T @ x (+bo), token-major layout ----
                NH = CHUNK // 512
                for ci in range(NCH):
                    x16 = x16_tiles[ci]
                    out_sb = spool.tile([128, 2, CHUNK], f16, tag="out_sb")
                    for oc in range(2):
                        for nh in range(NH):
                            ts_ = slice(nh * 512, (nh + 1) * 512)
                            o_ps = ps_post.tile([128, 512], f32, tag="o")
                            for ic in range(2):
                                nc.tensor.matmul(
                                    o_ps[:],
                                    lhsT=weff16[:, ic, oc * 128 : (oc + 1) * 128],
                                    rhs=x16[:, ic, ts_],
                                    start=(ic == 0),
                                    stop=(ic == 1),
                                )
                            # bias+copy: alternate ACT / DVE to balance engines
                            if nh % 2 == 0:
                                nc.scalar.activation(
                                    out_sb[:, oc, ts_],
                                    o_ps[:],
                                    AF.Identity,
                                    bias=bo_sb[:, oc : oc + 1],
                                )
                            else:
                                nc.vector.tensor_scalar_add(
                                    out_sb[:, oc, ts_],
                                    o_ps[:],
                                    bo_sb[:, oc : oc + 1],
                                )
                    nc.sync.dma_start(
                        out_r[:, :, ci * CHUNK : (ci + 1) * CHUNK], out_sb[:]
                    )

    nc.compile()
    return nc


def _get_nc():
    if "nc" not in _CACHE:
        _CACHE["nc"] = _build_nc()
    return _CACHE["nc"]


def kernel(**inputs) -> np.ndarray:
    global LAST_RESULTS
    from concourse.bass_utils import run_bass_kernel_spmd

    f8 = ml_dtypes.float8_e4m3
    x = np.ascontiguousarray(np.asarray(inputs["x"], dtype=np.float32))
    ctx = np.ascontiguousarray(np.asarray(inputs["context"], dtype=np.float32))
    Wq = np.asarray(inputs["Wq"], dtype=np.float32)
    Wk = np.asarray(inputs["Wk"], dtype=np.float32)
    Wv = np.asarray(inputs["Wv"], dtype=np.float32)
    Wo = np.asarray(inputs["Wo"], dtype=np.float32)
    bo = np.ascontiguousarray(
        np.asarray(inputs["bo"], dtype=np.float32).reshape(C, 1)
    )
    wkv8 = np.ascontiguousarray(
        np.concatenate([Wk, Wv], axis=1).astype(f8)
    )
    wqt16 = np.ascontiguousarray(Wq.T.astype(np.float16))
    wo16 = np.ascontiguousarray(Wo.astype(np.float16))

    x16 = x.reshape(B, C, N_FULL).astype(np.float16)
    c8 = ctx.reshape(B, C, N_FULL).astype(f8)

    in_maps = []
    for c in range(NCORES):
        b, t = c // 2, c % 2
        sl = slice(t * T, (t + 1) * T)
        in_maps.append(
            {
                "xh": np.ascontiguousarray(x16[b, :, sl]),
                "ch": np.ascontiguousarray(c8[b, :, sl]),
                "wkv": wkv8,
                "wqt": wqt16,
                "wo": wo16,
                "bo": bo,
            }
        )

    nc = _get_nc()
    res = run_bass_kernel_spmd(nc, in_maps, list(range(NCORES)), trace=TRACE)
    LAST_RESULTS = res

    out = np.empty((B, C, N_FULL), dtype=np.float32)
    for c in range(NCORES):
        b, t = c // 2, c % 2
        out[b, :, t * T : (t + 1) * T] = res.results[c]["out"].astype(np.float32)
    return out.reshape(B, C, 128, 128)


# revision 10
# speedup vs baseline: 1.2342x; 1.1508x over previous
"""Trainium2 Bass kernel for nn_CrossAttention (linear/efficient attention).

Math: out = x + reshape( x_flat @ W_eff + bo ) where
  W_eff = I + sum_h Wq_h @ cm_h @ Wo_h,
  cm_h  = softmax_n(k_h)^T @ v_h,  k = ctx_flat @ Wk, v = ctx_flat @ Wv.
(The q projection folds into W_eff; the residual folds in as the identity.)

Sharding: 8 cores = 4 batches x 2 token-halves. Each core computes partial
[num|den] softmax statistics over its 8192 tokens; a pairwise AllReduce
merges them; each core then applies W_eff to its own token half.

v2: phase-1 projections and cm accumulation run in fp8 (DoubleRow perf
mode, 256-deep contraction per pass); ctx arrives fp8 and x fp16 from the
host; the output is stored fp16 and widened on the host. Phase 2 stays
fp16 end to end so the residual identity inside W_eff keeps x's accuracy.
"""

import sys

if "/opt/trn_rl_repo" not in sys.path:
    sys.path.insert(0, "/opt/trn_rl_repo")

import numpy as np
import ml_dtypes

B = 4
C = 256          # channels (DIM)
N_FULL = 16384   # tokens per batch (128*128)
T = 8192         # tokens per core
HEADS = 8
DH = 64
INNER = 512
NCORES = 8
CHUNK = 2048
NCH = T // CHUNK      # 4
SUBS = CHUNK // 128   # 16

_CACHE: dict = {}
LAST_RESULTS = None   # BassKernelResults of the most recent run (for profiling)
TRACE = False         # set True before calling kernel() to capture a trace


def _build_nc():
    import concourse.mybir as mybir
    import concourse.tile as tile
    from concourse import bacc
    from concourse.masks import make_identity

    f32, f16, f8 = mybir.dt.float32, mybir.dt.float16, mybir.dt.float8e4
    AF = mybir.ActivationFunctionType
    DR = mybir.MatmulPerfMode.DoubleRow

    nc = bacc.Bacc("TRN2", target_bir_lowering=False, debug=False)

    xh = nc.dram_tensor("xh", [C, T], f16, kind="ExternalInput")
    ch = nc.dram_tensor("ch", [C, T], f8, kind="ExternalInput")
    wkv = nc.dram_tensor("wkv", [C, 2 * INNER], f8, kind="ExternalInput")
    wqt = nc.dram_tensor("wqt", [INNER, C], f16, kind="ExternalInput")
    wo = nc.dram_tensor("wo", [INNER, C], f16, kind="ExternalInput")
    bo = nc.dram_tensor("bo", [C, 1], f32, kind="ExternalInput")
    out = nc.dram_tensor("out", [C, T], f16, kind="ExternalOutput")

    xh_r = xh.ap().rearrange("(kc p) n -> p kc n", p=128)
    ch_r = ch.ap().rearrange("(kc p) n -> p kc n", p=128)
    out_r = out.ap().rearrange("(oc p) n -> p oc n", p=128)

    with tile.TileContext(nc) as tc:
        with (
            tc.tile_pool(name="wpool", bufs=1) as wpool,
            tc.tile_pool(name="spool", bufs=3) as spool,
            tc.tile_pool(name="ppool", bufs=4) as ppool,
            tc.tile_pool(name="x16pool", bufs=1) as x16pool,
            tc.tile_pool(name="dpool", bufs=1, space="DRAM") as dpool,
        ):
            def load_ctx8(ci):
                ctx8 = spool.tile([128, 2, CHUNK], f8, tag="ctx8", name="ctx8")
                nc.sync.dma_start(
                    ctx8[:], ch_r[:, :, ci * CHUNK : (ci + 1) * CHUNK]
                )
                return ctx8

            ctx8_next = load_ctx8(0)

            # ---- weights (already converted on host) ----
            wkv8 = wpool.tile([128, 2, 2 * INNER], f8)
            nc.sync.dma_start(wkv8[:], wkv.ap().rearrange("(kc p) o -> p kc o", p=128))
            wqt16 = wpool.tile([128, 4, C], f16)
            nc.sync.dma_start(
                wqt16[:], wqt.ap().rearrange("(hc p) i -> p hc i", p=128)
            )
            wo16 = wpool.tile([64, HEADS, C], f16)
            nc.sync.dma_start(wo16[:], wo.ap().rearrange("(h p) o -> p h o", p=64))
            bo_sb = wpool.tile([128, 2], f32)
            nc.sync.dma_start(bo_sb[:], bo.ap().rearrange("(oc p) x -> p (oc x)", p=128))
            ident16 = wpool.tile([128, 128], f16)
            make_identity(nc, ident16[:])

            # ---- phase 1: accumulate per-head [num | den] over local tokens ----
            # cm_ps[hp] rows 0:64   = head 2hp   : cols 0:64 num, col 64 den
            #           rows 64:128 = head 2hp+1 : cols 65:129 num, col 129 den
            cm_sb = wpool.tile([128, 4, 130], f32)
            x16_tiles = []

            with (
                tc.tile_pool(name="ps_cm", bufs=1, space="PSUM") as ps_cm,
                tc.tile_pool(name="ps_kv", bufs=2, space="PSUM") as ps_kv,
            ):
                cm_ps = [
                    ps_cm.tile([128, 130], f32, tag=f"cm{i}", name=f"cm{i}")
                    for i in range(4)
                ]
                for ci in range(NCH):
                    ctx8 = ctx8_next
                    if ci + 1 < NCH:
                        ctx8_next = load_ctx8(ci + 1)
                    # phase-2 x tile: f16 straight from HBM, kept resident
                    x16 = x16pool.tile(
                        [128, 2, CHUNK], f16, tag=f"x16_{ci}", name=f"x16_{ci}"
                    )
                    nc.sync.dma_start(
                        x16[:], xh_r[:, :, ci * CHUNK : (ci + 1) * CHUNK]
                    )
                    x16_tiles.append(x16)

                    for s in range(SUBS):
                        tok = slice(s * 128, (s + 1) * 128)
                        half = s % 2
                        if half == 0:
                            kexp8 = ppool.tile([128, 2, INNER], f8, tag="kexp")
                            vcat8 = ppool.tile([128, 2, 8, 65], f8, tag="vcat")
                            nc.gpsimd.memset(vcat8[:, :, :, 64], 1.0)
                        # K and V projections: contraction 256 in one
                        # DoubleRow pass each (PSUM bank limits out to 512 f32)
                        k_ps = ps_kv.tile([128, INNER], f32, tag="k")
                        nc.tensor.matmul(
                            k_ps[:],
                            lhsT=ctx8[:, :, tok],
                            rhs=wkv8[:, :, 0:INNER],
                            start=True,
                            stop=True,
                            perf_mode=DR,
                        )
                        v_ps = ps_kv.tile([128, INNER], f32, tag="v")
                        nc.tensor.matmul(
                            v_ps[:],
                            lhsT=ctx8[:, :, tok],
                            rhs=wkv8[:, :, INNER : 2 * INNER],
                            start=True,
                            stop=True,
                            perf_mode=DR,
                        )
                        nc.scalar.activation(
                            kexp8[:, half, :], k_ps[:], AF.Exp
                        )
                        nc.vector.tensor_copy(
                            vcat8[:, half, :, 0:64],
                            v_ps[:].rearrange("p (h e) -> p h e", h=8),
                        )
                        if half == 1:
                            first = ci == 0 and s == 1
                            last = ci == NCH - 1 and s == SUBS - 1
                            for hp in range(4):
                                nc.tensor.matmul(
                                    cm_ps[hp][:],
                                    lhsT=kexp8[:, :, hp * 128 : (hp + 1) * 128],
                                    rhs=vcat8[:, :, 2 * hp : 2 * hp + 2, :],
                                    start=first,
                                    stop=last,
                                    perf_mode=DR,
                                )
                for hp in range(4):
                    nc.vector.tensor_copy(cm_sb[:, hp, :], cm_ps[hp][:])

            # ---- pairwise AllReduce of [num|den] across the 2 token halves ----
            # trim to the useful halves: head h -> rows (h%2)*64, chunk h//2,
            # cols 0:64 num, col 64 den
            cc_in = dpool.tile([128, 4, 65], f32)
            cc_out = dpool.tile([128, 4, 65], f32)
            nc.sync.dma_start(cc_in[0:64, :, :], cm_sb[0:64, :, 0:65])
            nc.sync.dma_start(cc_in[64:128, :, :], cm_sb[64:128, :, 65:130])
            nc.gpsimd.collective_compute(
                "AllReduce",
                mybir.AluOpType.add,
                replica_groups=[[0, 1], [2, 3], [4, 5], [6, 7]],
                ins=[cc_in.opt()],
                outs=[cc_out.opt()],
            )
            mm_sb = wpool.tile([128, 4, 65], f32)
            nc.sync.dma_start(mm_sb[:], cc_out[:])

            # keep the PE clock warm through the AllReduce window: small
            # matmuls that drain fast once real work arrives
            with (
                tc.tile_pool(name="ps_warm", bufs=1, space="PSUM") as ps_warm,
                tc.tile_pool(name="ps_post", bufs=2, space="PSUM") as ps_post,
            ):
                warm_ps = ps_warm.tile([128, 2 * C], f32, tag="warm", name="warm_ps")
                for _ in range(100):
                    nc.tensor.matmul(
                        warm_ps[:],
                        lhsT=wqt16[:, 0, 0:128],
                        rhs=wqt16[:, 0:2, :],
                        start=True,
                        stop=True,
                    )

                # ---- normalize cm, build W_eff = I + sum_h Wq_h cm_h Wo_h ----
                deninv = wpool.tile([128, 4], f32)
                cmn16 = wpool.tile([128, 4, 64], f16)
                m1t16 = wpool.tile([64, 8, C], f16)
                weff16 = wpool.tile([128, 2, C], f16)
                nc.vector.reciprocal(deninv[:], mm_sb[:, :, 64])
                for hp in range(4):
                    nc.vector.tensor_scalar_mul(
                        cmn16[:, hp, :],
                        mm_sb[:, hp, 0:64],
                        deninv[:, hp : hp + 1],
                    )
                for h in range(HEADS):
                    hp, hh = h // 2, h % 2
                    rs = slice(hh * 64, hh * 64 + 64)
                    m1t_ps = ps_post.tile([64, C], f32, tag="m1t")
                    nc.tensor.matmul(
                        m1t_ps[:],
                        lhsT=cmn16[rs, hp, :],
                        rhs=wqt16[rs, hp, :],
                        start=True,
                        stop=True,
                    )
                    nc.vector.tensor_copy(m1t16[:, h, :], m1t_ps[:])
                for ic in range(2):
                    weff_ps = ps_post.tile([128, C], f32, tag="weff")
                    for h in range(HEADS):
                        nc.tensor.matmul(
                            weff_ps[:],
                            lhsT=m1t16[:, h, ic * 128 : (ic + 1) * 128],
                            rhs=wo16[:, h, :],
                            start=(h == 0),
                            stop=False,
                        )
                    # fold the residual in: W_eff += I (this core's row block)
                    nc.tensor.matmul(
                        weff_ps[:, ic * 128 : (ic + 1) * 128],
                        lhsT=ident16[:],
                        rhs=ident16[:],
                        start=False,
                        stop=True,
                    )
                    nc.vector.tensor_copy(weff16[:, ic, :], weff_ps[:])

                # ---- phase 2: out = W_eff^T @ x (+bo), token-major layout ----
                NH = CHUNK // 512
                for ci in range(NCH):
                    x16 = x16_tiles[ci]
                    out_sb = spool.tile([128, 2, CHUNK], f16, tag="out_sb")
                    for oc in range(2):
                        for nh in range(NH):
                            ts_ = slice(nh * 512, (nh + 1) * 512)
                            o_ps = ps_post.tile([128, 512], f32, tag="o")
                            for ic in range(2):
                                nc.tensor.matmul(
                                    o_ps[:],
                                    lhsT=weff16[:, ic, oc * 128 : (oc + 1) * 128],
                                    rhs=x16[:, ic, ts_],
                                    start=(ic == 0),
                                    stop=(ic == 1),
                                )
                            # bias+copy: alternate ACT / DVE to balance engines
                            if nh % 2 == 0:
                                nc.scalar.activation(
                                    out_sb[:, oc, ts_],
                                    o_ps[:],
                                    AF.Identity,
                                    bias=bo_sb[:, oc : oc + 1],
                                )
                            else:
                                nc.vector.tensor_scalar_add(
                                    out_sb[:, oc, ts_],
                                    o_ps[:],
                                    bo_sb[:, oc : oc + 1],
                                )
                    nc.sync.dma_start(
                        out_r[:, :, ci * CHUNK : (ci + 1) * CHUNK], out_sb[:]
                    )

    nc.compile()
    return nc


def _get_nc():
    if "nc" not in _CACHE:
        _CACHE["nc"] = _build_nc()
    return _CACHE["nc"]


def kernel(**inputs) -> np.ndarray:
    global LAST_RESULTS
    from concourse.bass_utils import run_bass_kernel_spmd

    f8 = ml_dtypes.float8_e4m3
    x = np.ascontiguousarray(np.asarray(inputs["x"], dtype=np.float32))
    ctx = np.ascontiguousarray(np.asarray(inputs["context"], dtype=np.float32))
    Wq = np.asarray(inputs["Wq"], dtype=np.float32)
    Wk = np.asarray(inputs["Wk"], dtype=np.float32)
    Wv = np.asarray(inputs["Wv"], dtype=np.float32)
    Wo = np.asarray(inputs["Wo"], dtype=np.float32)
    bo = np.ascontiguousarray(
        np.asarray(inputs["bo"], dtype=np.float32).reshape(C, 1)
    )
    wkv8 = np.ascontiguousarray(
        np.concatenate([Wk, Wv], axis=1).astype(f8)
    )
    wqt16 = np.ascontiguousarray(Wq.T.astype(np.float16))
    wo16 = np.ascontiguousarray(Wo.astype(np.float16))

    x16 = x.reshape(B, C, N_FULL).astype(np.float16)
    c8 = ctx.reshape(B, C, N_FULL).astype(f8)

    in_maps = []
    for c in range(NCORES):
        b, t = c // 2, c % 2
        sl = slice(t * T, (t + 1) * T)
        in_maps.append(
            {
                "xh": np.ascontiguousarray(x16[b, :, sl]),
                "ch": np.ascontiguousarray(c8[b, :, sl]),
                "wkv": wkv8,
                "wqt": wqt16,
                "wo": wo16,
                "bo": bo,
            }
        )

    nc = _get_nc()
    res = run_bass_kernel_spmd(nc, in_maps, list(range(NCORES)), trace=TRACE)
    LAST_RESULTS = res

    out = np.empty((B, C, N_FULL), dtype=np.float32)
    for c in range(NCORES):
        b, t = c // 2, c % 2
        out[b, :, t * T : (t + 1) * T] = res.results[c]["out"].astype(np.float32)
    return out.reshape(B, C, 128, 128)


# revision 11
# speedup vs baseline: 1.2786x; 1.0360x over previous
"""Trainium2 Bass kernel for nn_CrossAttention (linear/efficient attention).

Math: out = x + reshape( x_flat @ W_eff + bo ) where
  W_eff = I + sum_h Wq_h @ cm_h @ Wo_h,
  cm_h  = softmax_n(k_h)^T @ v_h,  k = ctx_flat @ Wk, v = ctx_flat @ Wv.
(The q projection folds into W_eff; the residual folds in as the identity.)

Sharding: 8 cores = 4 batches x 2 token-halves. Each core computes partial
[num|den] softmax statistics over its 8192 tokens; a pairwise AllReduce
merges them; each core then applies W_eff to its own token half.

v2: phase-1 projections and cm accumulation run in fp8 (DoubleRow perf
mode, 256-deep contraction per pass); ctx arrives fp8 and x fp16 from the
host; the output is stored fp16 and widened on the host. Phase 2 stays
fp16 end to end so the residual identity inside W_eff keeps x's accuracy.
"""

import sys

if "/opt/trn_rl_repo" not in sys.path:
    sys.path.insert(0, "/opt/trn_rl_repo")

import numpy as np
import ml_dtypes

B = 4
C = 256          # channels (DIM)
N_FULL = 16384   # tokens per batch (128*128)
T = 8192         # tokens per core
HEADS = 8
DH = 64
INNER = 512
NCORES = 8
CHUNK = 2048
NCH = T // CHUNK      # 4
SUBS = CHUNK // 128   # 16

_CACHE: dict = {}
LAST_RESULTS = None   # BassKernelResults of the most recent run (for profiling)
TRACE = False         # set True before calling kernel() to capture a trace


def _build_nc():
    import concourse.mybir as mybir
    import concourse.tile as tile
    from concourse import bacc
    from concourse.masks import make_identity

    f32, f16, f8 = mybir.dt.float32, mybir.dt.float16, mybir.dt.float8e4
    AF = mybir.ActivationFunctionType
    DR = mybir.MatmulPerfMode.DoubleRow

    nc = bacc.Bacc("TRN2", target_bir_lowering=False, debug=False)

    xh = nc.dram_tensor("xh", [C, T], f16, kind="ExternalInput")
    ch = nc.dram_tensor("ch", [C, T], f8, kind="ExternalInput")
    wkv = nc.dram_tensor("wkv", [C, 2 * INNER], f8, kind="ExternalInput")
    wqt = nc.dram_tensor("wqt", [INNER, C], f16, kind="ExternalInput")
    wo = nc.dram_tensor("wo", [INNER, C], f16, kind="ExternalInput")
    bo = nc.dram_tensor("bo", [C, 1], f32, kind="ExternalInput")
    out = nc.dram_tensor("out", [C, T], f16, kind="ExternalOutput")

    xh_r = xh.ap().rearrange("(kc p) n -> p kc n", p=128)
    ch_r = ch.ap().rearrange("(kc p) n -> p kc n", p=128)
    out_r = out.ap().rearrange("(oc p) n -> p oc n", p=128)

    with tile.TileContext(nc) as tc:
        with (
            tc.tile_pool(name="wpool", bufs=1) as wpool,
            tc.tile_pool(name="spool", bufs=3) as spool,
            tc.tile_pool(name="ppool", bufs=4) as ppool,
            tc.tile_pool(name="x16pool", bufs=1) as x16pool,
            tc.tile_pool(name="dpool", bufs=1, space="DRAM") as dpool,
        ):
            def load_ctx8(ci):
                ctx8 = spool.tile([128, 2, CHUNK], f8, tag="ctx8", name="ctx8")
                nc.sync.dma_start(
                    ctx8[:], ch_r[:, :, ci * CHUNK : (ci + 1) * CHUNK]
                )
                return ctx8

            ctx8_next = load_ctx8(0)

            # ---- weights (already converted on host) ----
            wkv8 = wpool.tile([128, 2, 2 * INNER], f8)
            nc.sync.dma_start(wkv8[:], wkv.ap().rearrange("(kc p) o -> p kc o", p=128))
            wqt16 = wpool.tile([128, 4, C], f16)
            nc.sync.dma_start(
                wqt16[:], wqt.ap().rearrange("(hc p) i -> p hc i", p=128)
            )
            wo16 = wpool.tile([64, HEADS, C], f16)
            nc.sync.dma_start(wo16[:], wo.ap().rearrange("(h p) o -> p h o", p=64))
            bo_sb = wpool.tile([128, 2], f32)
            nc.sync.dma_start(bo_sb[:], bo.ap().rearrange("(oc p) x -> p (oc x)", p=128))
            ident16 = wpool.tile([128, 128], f16)
            make_identity(nc, ident16[:])

            # tiny dummy AllReduce issued up front: pays the collective
            # channel-setup latency while phase 1 runs
            ccw_in = dpool.tile([128, 4], f32)
            ccw_out = dpool.tile([128, 4], f32)
            nc.sync.dma_start(ccw_in[:, 0:2], bo_sb[:])
            nc.sync.dma_start(ccw_in[:, 2:4], bo_sb[:])
            nc.gpsimd.collective_compute(
                "AllReduce",
                mybir.AluOpType.add,
                replica_groups=[[0, 1], [2, 3], [4, 5], [6, 7]],
                ins=[ccw_in.opt()],
                outs=[ccw_out.opt()],
            )

            # ---- phase 1: accumulate per-head [num | den] over local tokens ----
            # cm_ps[hp] rows 0:64   = head 2hp   : cols 0:64 num, col 64 den
            #           rows 64:128 = head 2hp+1 : cols 65:129 num, col 129 den
            cm_sb = wpool.tile([128, 4, 130], f32)
            x16_tiles = []

            with (
                tc.tile_pool(name="ps_cm", bufs=1, space="PSUM") as ps_cm,
                tc.tile_pool(name="ps_kv", bufs=2, space="PSUM") as ps_kv,
            ):
                cm_ps = [
                    ps_cm.tile([128, 130], f32, tag=f"cm{i}", name=f"cm{i}")
                    for i in range(4)
                ]
                for ci in range(NCH):
                    ctx8 = ctx8_next
                    if ci + 1 < NCH:
                        ctx8_next = load_ctx8(ci + 1)
                    # phase-2 x tile: f16 straight from HBM, kept resident
                    x16 = x16pool.tile(
                        [128, 2, CHUNK], f16, tag=f"x16_{ci}", name=f"x16_{ci}"
                    )
                    nc.sync.dma_start(
                        x16[:], xh_r[:, :, ci * CHUNK : (ci + 1) * CHUNK]
                    )
                    x16_tiles.append(x16)

                    for s in range(SUBS):
                        tok = slice(s * 128, (s + 1) * 128)
                        half = s % 2
                        if half == 0:
                            kexp8 = ppool.tile([128, 2, INNER], f8, tag="kexp")
                            vcat8 = ppool.tile([128, 2, 8, 65], f8, tag="vcat")
                            nc.gpsimd.memset(vcat8[:, :, :, 64], 1.0)
                        # K and V projections: contraction 256 in one
                        # DoubleRow pass each (PSUM bank limits out to 512 f32)
                        k_ps = ps_kv.tile([128, INNER], f32, tag="k")
                        nc.tensor.matmul(
                            k_ps[:],
                            lhsT=ctx8[:, :, tok],
                            rhs=wkv8[:, :, 0:INNER],
                            start=True,
                            stop=True,
                            perf_mode=DR,
                        )
                        v_ps = ps_kv.tile([128, INNER], f32, tag="v")
                        nc.tensor.matmul(
                            v_ps[:],
                            lhsT=ctx8[:, :, tok],
                            rhs=wkv8[:, :, INNER : 2 * INNER],
                            start=True,
                            stop=True,
                            perf_mode=DR,
                        )
                        nc.scalar.activation(
                            kexp8[:, half, :], k_ps[:], AF.Exp
                        )
                        nc.vector.tensor_copy(
                            vcat8[:, half, :, 0:64],
                            v_ps[:].rearrange("p (h e) -> p h e", h=8),
                        )
                        if half == 1:
                            first = ci == 0 and s == 1
                            last = ci == NCH - 1 and s == SUBS - 1
                            for hp in range(4):
                                nc.tensor.matmul(
                                    cm_ps[hp][:],
                                    lhsT=kexp8[:, :, hp * 128 : (hp + 1) * 128],
                                    rhs=vcat8[:, :, 2 * hp : 2 * hp + 2, :],
                                    start=first,
                                    stop=last,
                                    perf_mode=DR,
                                )
                for hp in range(4):
                    nc.vector.tensor_copy(cm_sb[:, hp, :], cm_ps[hp][:])

            # ---- pairwise AllReduce of [num|den] across the 2 token halves ----
            # trim to the useful halves: head h -> rows (h%2)*64, chunk h//2,
            # cols 0:64 num, col 64 den
            cc_in = dpool.tile([128, 4, 65], f32)
            cc_out = dpool.tile([128, 4, 65], f32)
            nc.sync.dma_start(cc_in[0:64, :, :], cm_sb[0:64, :, 0:65])
            nc.sync.dma_start(cc_in[64:128, :, :], cm_sb[64:128, :, 65:130])
            nc.gpsimd.collective_compute(
                "AllReduce",
                mybir.AluOpType.add,
                replica_groups=[[0, 1], [2, 3], [4, 5], [6, 7]],
                ins=[cc_in.opt()],
                outs=[cc_out.opt()],
            )
            mm_sb = wpool.tile([128, 4, 65], f32)
            nc.sync.dma_start(mm_sb[:], cc_out[:])

            # keep the PE clock warm through the AllReduce window: small
            # matmuls that drain fast once real work arrives
            with tc.tile_pool(name="ps_warm", bufs=1, space="PSUM") as ps_warm:
                warm_ps = ps_warm.tile(
                    [128, 2 * C], f32, tag="warm", name="warm_ps"
                )
                for _ in range(30):
                    nc.tensor.matmul(
                        warm_ps[:],
                        lhsT=wqt16[:, 0, 0:128],
                        rhs=wqt16[:, 0:2, :],
                        start=True,
                        stop=True,
                    )
            with (
                tc.tile_pool(name="ps_post", bufs=2, space="PSUM") as ps_post,
                tc.tile_pool(name="ps_o", bufs=4, space="PSUM") as ps_o,
            ):

                # ---- normalize cm, build W_eff = I + sum_h Wq_h cm_h Wo_h ----
                deninv = wpool.tile([128, 4], f32)
                cmn16 = wpool.tile([128, 4, 64], f16)
                m1t16 = wpool.tile([64, 8, C], f16)
                weff16 = wpool.tile([128, 2, C], f16)
                nc.vector.reciprocal(deninv[:], mm_sb[:, :, 64])
                for hp in range(4):
                    nc.vector.tensor_scalar_mul(
                        cmn16[:, hp, :],
                        mm_sb[:, hp, 0:64],
                        deninv[:, hp : hp + 1],
                    )
                for h in range(HEADS):
                    hp, hh = h // 2, h % 2
                    rs = slice(hh * 64, hh * 64 + 64)
                    m1t_ps = ps_post.tile([64, C], f32, tag="m1t")
                    nc.tensor.matmul(
                        m1t_ps[:],
                        lhsT=cmn16[rs, hp, :],
                        rhs=wqt16[rs, hp, :],
                        start=True,
                        stop=True,
                    )
                    nc.vector.tensor_copy(m1t16[:, h, :], m1t_ps[:])
                for ic in range(2):
                    weff_ps = ps_post.tile([128, C], f32, tag="weff")
                    for h in range(HEADS):
                        nc.tensor.matmul(
                            weff_ps[:],
                            lhsT=m1t16[:, h, ic * 128 : (ic + 1) * 128],
                            rhs=wo16[:, h, :],
                            start=(h == 0),
                            stop=False,
                        )
                    # fold the residual in: W_eff += I (this core's row block)
                    nc.tensor.matmul(
                        weff_ps[:, ic * 128 : (ic + 1) * 128],
                        lhsT=ident16[:],
                        rhs=ident16[:],
                        start=False,
                        stop=True,
                    )
                    nc.vector.tensor_copy(weff16[:, ic, :], weff_ps[:])

                # ---- phase 2: out = W_eff^T @ x (+bo), token-major layout ----
                NH = CHUNK // 512
                for ci in range(NCH):
                    x16 = x16_tiles[ci]
                    out_sb = spool.tile([128, 2, CHUNK], f16, tag="out_sb")
                    for oc in range(2):
                        for nh in range(NH):
                            ts_ = slice(nh * 512, (nh + 1) * 512)
                            o_ps = ps_o.tile([128, 512], f32, tag="o")
                            for ic in range(2):
                                nc.tensor.matmul(
                                    o_ps[:],
                                    lhsT=weff16[:, ic, oc * 128 : (oc + 1) * 128],
                                    rhs=x16[:, ic, ts_],
                                    start=(ic == 0),
                                    stop=(ic == 1),
                                )
                            # bias+copy: alternate ACT / DVE to balance engines
                            if nh % 2 == 0:
                                nc.scalar.activation(
                                    out_sb[:, oc, ts_],
                                    o_ps[:],
                                    AF.Identity,
                                    bias=bo_sb[:, oc : oc + 1],
                                )
                            else:
                                nc.vector.tensor_scalar_add(
                                    out_sb[:, oc, ts_],
                                    o_ps[:],
                                    bo_sb[:, oc : oc + 1],
                                )
                    nc.sync.dma_start(
                        out_r[:, :, ci * CHUNK : (ci + 1) * CHUNK], out_sb[:]
                    )

    nc.compile()
    return nc


def _get_nc():
    if "nc" not in _CACHE:
        _CACHE["nc"] = _build_nc()
    return _CACHE["nc"]


def kernel(**inputs) -> np.ndarray:
    global LAST_RESULTS
    from concourse.bass_utils import run_bass_kernel_spmd

    f8 = ml_dtypes.float8_e4m3
    x = np.ascontiguousarray(np.asarray(inputs["x"], dtype=np.float32))
    ctx = np.ascontiguousarray(np.asarray(inputs["context"], dtype=np.float32))
    Wq = np.asarray(inputs["Wq"], dtype=np.float32)
    Wk = np.asarray(inputs["Wk"], dtype=np.float32)
    Wv = np.asarray(inputs["Wv"], dtype=np.float32)
    Wo = np.asarray(inputs["Wo"], dtype=np.float32)
    bo = np.ascontiguousarray(
        np.asarray(inputs["bo"], dtype=np.float32).reshape(C, 1)
    )
    wkv8 = np.ascontiguousarray(
        np.concatenate([Wk, Wv], axis=1).astype(f8)
    )
    wqt16 = np.ascontiguousarray(Wq.T.astype(np.float16))
    wo16 = np.ascontiguousarray(Wo.astype(np.float16))

    x16 = x.reshape(B, C, N_FULL).astype(np.float16)
    c8 = ctx.reshape(B, C, N_FULL).astype(f8)

    in_maps = []
    for c in range(NCORES):
        b, t = c // 2, c % 2
        sl = slice(t * T, (t + 1) * T)
        in_maps.append(
            {
                "xh": np.ascontiguousarray(x16[b, :, sl]),
                "ch": np.ascontiguousarray(c8[b, :, sl]),
                "wkv": wkv8,
                "wqt": wqt16,
                "wo": wo16,
                "bo": bo,
            }
        )

    nc = _get_nc()
    res = run_bass_kernel_spmd(nc, in_maps, list(range(NCORES)), trace=TRACE)
    LAST_RESULTS = res

    out = np.empty((B, C, N_FULL), dtype=np.float32)
    for c in range(NCORES):
        b, t = c // 2, c % 2
        out[b, :, t * T : (t + 1) * T] = res.results[c]["out"].astype(np.float32)
    return out.reshape(B, C, 128, 128)
